# revision 32
# baseline (speedup 1.0000x reference)
"""Trainium2 Bass kernel for nn_LogicalAttentionLayer (per-token cross-head attention).

Math (per token t):
  q,k,v = x @ W{q,k,v}.T + b   -> [NH=16, HD=64] per token
  scores[h,g] = (q_h . k_g) / 8 ; attn = softmax_g(scores)
  u[h,:] = sum_g attn[h,g] * v[g,:] ; y = u_flat @ Wo.T + bo

Sharding: data-parallel over the 16384 tokens -> 2048 tokens per core x 8 cores.
Host-side input marshalling: x^T and W^T are pre-transposed and pre-cast to
bf16 per core (layout prep, so the device spends no DMA/engine time on
dtype-cast round trips or xbar transposes of inputs).

Per-core plan (everything contraction-shaped runs on the PE):
  - prologue: single DMA loads of x^T, W^T (bf16) into SBUF; bias rows
    broadcast to [128, 1024] tiles via rank-1 PE matmul so bias is added
    during the psum drain; identity + block-diag mask constants.
  - projections: psum[t,512] = sum_ic x^T[ic,:].T @ W^T[ic,:], DVE drains
    psum + bias -> bf16 SBUF.
  - attention, batched 8 tokens per PE matmul ("group", 16 groups per
    128-token tile; scores phase then attn phase so PE never waits on
    ACT/DVE between small matmuls):
      qT,kT[d, t*16+h] built with PE transposes of 64-wide head chunks;
      scoresT psum[(t,g),(t,h)] = kT.T @ qT  (K=64)
      eT = exp(scoresT/8) on ACT; block-diag mask by Pool multiply
      V' [(t,g), d|1] built by one DMA per tile via a DRAM round trip
      uaug[(t,h), d|den] = eT_masked.T @ V'  (K=128)
      DVE: recip(den), u = uaug * recip  -> u_all, one DMA per tile to DRAM
  - y = u @ Wo.T + bo: dma-transpose u^T from DRAM (issued one tile early so
    the transfer hides behind the next tile), PE matmuls, fp32 out.
"""

import os
import sys

# The tile framework's optimize_sems pass elides DMA completion semaphores
# by assuming per-engine FIFO execution of HWDGE DMAs. Under the
# bass2jax/walrus NEFF path the DMA-queue assignment does not honor that
# assumption (observed nondeterministic stale reads of DRAM scratch), so
# keep every DMA semaphore explicit.
os.environ["BACC_ELIDE_DMA_OPT_LIMIT"] = "0"

for p in ("/opt/trn_rl_repo",):
    if p not in sys.path:
        sys.path.insert(0, p)

import numpy as np

import concourse.bass as bass
import concourse.mybir as mybir
from concourse import bacc
from concourse.bass_utils import run_bass_kernel_spmd
from concourse.tile import TileContext

NCORES = 8
B, S, HID, NH, HD = 4, 4096, 1024, 16, 64
T_FULL = B * S
T = T_FULL // NCORES          # 2048 tokens per core
NT = T // 128                 # 16 token tiles
IC = HID // 128               # 8 contraction chunks
NG = 16                       # 8-token groups per 128-token tile
BF = mybir.dt.bfloat16
F32 = mybir.dt.float32
I32 = mybir.dt.int32
OP = mybir.AluOpType
AF = mybir.ActivationFunctionType

NP_BF16 = mybir.dt.np(BF)

_cached = None


def build_program(reps=1):
    nc = bacc.Bacc("TRN2")

    xT = nc.dram_tensor("xT", [HID, T], BF, kind="ExternalInput")
    WT = {n: nc.dram_tensor(f"{n}T", [HID, HID], BF, kind="ExternalInput")
          for n in ("Wq", "Wk", "Wv", "Wo")}
    bias = {n: nc.dram_tensor(n, [HID], F32, kind="ExternalInput")
            for n in ("bq", "bk", "bv", "bo")}
    ident_in = nc.dram_tensor("ident", [128, 128], BF, kind="ExternalInput")
    mask_in = nc.dram_tensor("mask4", [128, 512], BF, kind="ExternalInput")
    y = nc.dram_tensor("y", [T, HID], F32, kind="ExternalOutput")

    with TileContext(nc) as tc:
        with (
            tc.tile_pool(name="dram", bufs=1, space="DRAM") as dp,
            tc.tile_pool(name="persist", bufs=1) as pp,
            tc.tile_pool(name="qkv", bufs=2) as qp,
            tc.tile_pool(name="tpsb", bufs=2) as tp_sb,
            tc.tile_pool(name="vall", bufs=2) as vp,
            tc.tile_pool(name="attn", bufs=2) as ap_,
            tc.tile_pool(name="em4", bufs=5) as ep_,
            tc.tile_pool(name="uall", bufs=2) as up,
            tc.tile_pool(name="out", bufs=2) as op_,
            tc.tile_pool(name="mmps", bufs=2, space="PSUM") as mmps,
            tc.tile_pool(name="yps", bufs=2, space="PSUM") as yps,
            tc.tile_pool(name="tps", bufs=2, space="PSUM") as tps,
            tc.tile_pool(name="sps", bufs=2, space="PSUM") as sps,
        ):
            # ---------------- prologue: direct bf16 loads ----------------
            # order by first consumer: Wv, tile-0 slice of x^T (so tile 0
            # starts ~10us earlier), Wq, Wk, then the bulk of x^T, then Wo
            wt = {}
            xT4 = xT.rearrange("(ic p) t -> p ic t", p=128)

            def load_w(n):
                wtn = pp.tile([128, IC * HID], BF, name=f"wt{n}")
                nc.sync.dma_start(wtn[:, :],
                                  WT[n].rearrange("(ic p) o -> p ic o", p=128))
                wt[n] = wtn

            load_w("Wv")
            xt0 = pp.tile([128, IC * 128], BF, name="xt0")
            nc.sync.dma_start(xt0[:, :], xT4[:, :, 0:128])
            load_w("Wq")
            load_w("Wk")
            xt = pp.tile([128, IC * T], BF, name="xt")
            nc.sync.dma_start(xt[:, :], xT4)
            load_w("Wo")

            ones_sb = pp.tile([1, 128], BF, name="ones")
            nc.vector.memset(ones_sb[:, :], 1.0)

            # bias rows -> broadcast [128, HID] tiles (rank-1 PE matmul)
            brow = {}
            for n in ("bq", "bk", "bv", "bo"):
                bl = pp.tile([1, HID], F32, name=f"{n}f")
                nc.scalar.dma_start(bl[:, :],
                                    bias[n].rearrange("(a o) -> a o", a=1))
                bb = pp.tile([1, HID], BF, name=f"{n}b")
                nc.scalar.copy(bb[:, :], bl[:, :])
                brow[n] = bb
            btile = {}
            for n in ("bq", "bk", "bv", "bo"):
                bt = pp.tile([128, HID], F32 if n == "bo" else BF, name=f"{n}t")
                for hf in range(2):
                    p = mmps.tile([128, 512], F32, tag="proj", name="bproj")
                    nc.tensor.matmul(p[:, :], ones_sb[:, :],
                                     brow[n][:, hf * 512:(hf + 1) * 512],
                                     start=True, stop=True)
                    nc.scalar.copy(bt[:, hf * 512:(hf + 1) * 512], p[:, :])
                btile[n] = bt

            # identity (for PE transposes) and 4x block-diagonal mask
            # (mask4[p, n] = 1.0 iff p//16 == (n%128)//16) come from the host
            ident = pp.tile([128, 128], BF, name="ident")
            nc.sync.dma_start(ident[:, :], ident_in[:, :])
            mask4 = pp.tile([128, 512], BF, name="mask4")
            nc.sync.dma_start(mask4[:, :], mask_in[:, :])

            ubf = dp.tile([T, HID], BF, name="ubf")
            vdram = dp.tile([T, HID], BF, name="vdram")

            # 4D views for the batched V'/u DMAs (DRAM side is a flat
            # address stream, so the partition-crossing interleave is legal)
            v_src = vdram.rearrange("(tt j t) (g d) -> tt t g j d",
                                    j=NG, t=8, d=HD)
            u_dst = ubf.rearrange("(tt j t) (h d) -> tt t h j d",
                                  j=NG, t=8, d=HD)

            def yproj(gr):
                for tl in range(4):
                    tj = gr * 4 + tl
                    y_sb = op_.tile([128, HID], F32, tag="ysb", name="y_sb")
                    for hf in range(2):
                        yp = yps.tile([128, 512], F32, tag="yproj",
                                      name="yproj")
                        for ic in range(IC):
                            lt = uT_tiles[gr][:, ic * 512 + tl * 128:
                                              ic * 512 + (tl + 1) * 128]
                            nc.tensor.matmul(
                                yp[:, :], lt,
                                wt["Wo"][:, ic * HID + hf * 512:
                                         ic * HID + (hf + 1) * 512],
                                start=(ic == 0), stop=(ic == IC - 1))
                        nc.vector.tensor_tensor(
                            y_sb[:, hf * 512:(hf + 1) * 512], yp[:, :],
                            btile["bo"][:, hf * 512:(hf + 1) * 512], OP.add)
                    nc.sync.dma_start(y[tj * 128:(tj + 1) * 128, :],
                                      y_sb[:, :])

            uT_tiles = {}
            uTt_tiles = {}

            # ---------------- main loop over token tiles ----------------
            # reps > 1 repeats the whole token loop (benchmarking: the
            # wall-clock slope over reps isolates device loop time)
            def main_loop():
              for ti in range(NT * reps):
                ti = ti % NT
                # u^T transposes of the previous 4-tile block, one tile after
                # its last u write (margin for the same-ring DMA ordering)
                if ti % 4 == 0 and ti > 0 and ti // 4 - 1 < NT // 4 - 1:
                    issue_uT(ti // 4 - 1)
                # y-projection one tile after that (transfers done by then)
                if ti % 4 == 1 and ti > 1 and ti // 4 - 1 < NT // 4 - 1:
                    yproj(ti // 4 - 1)
                # last 4-tile block drains per tile so the epilogue is short
                if ti >= NT - 3:
                    issue_uT_tile(ti - 1)
                if ti >= NT - 2:
                    yproj_tile(ti - 2)

                # --- projections v,q,k -> SBUF bf16 [128, 1024] ---
                sb = {}
                for n, bn in (("Wv", "bv"), ("Wq", "bq"), ("Wk", "bk")):
                    dst = qp.tile([128, HID], BF, tag=n, name=f"{n}sb")
                    for hf in range(2):
                        p = mmps.tile([128, 512], F32, tag="proj", name="proj")
                        for ic in range(IC):
                            if ti == 0:
                                lt = xt0[:, ic * 128:(ic + 1) * 128]
                            else:
                                lt = xt[:, ic * T + ti * 128:
                                        ic * T + (ti + 1) * 128]
                            nc.tensor.matmul(
                                p[:, :], lt,
                                wt[n][:, ic * HID + hf * 512:
                                      ic * HID + (hf + 1) * 512],
                                start=(ic == 0), stop=(ic == IC - 1))
                        nc.vector.tensor_tensor(
                            dst[:, hf * 512:(hf + 1) * 512], p[:, :],
                            btile[bn][:, hf * 512:(hf + 1) * 512], OP.add)
                    sb[n] = dst
                    if n == "Wv":
                        # start the V' DRAM round trip as early as possible
                        nc.sync.dma_start(vdram[ti * 128:(ti + 1) * 128, :],
                                          dst[:, :])
                        Vall = vp.tile([128, NG * (HD + 1)], BF, tag="Vall",
                                       name="Vall")
                        Vall3 = Vall.rearrange("p (j c) -> p j c", c=HD + 1)
                        nc.vector.memset(Vall3[:, :, HD:HD + 1], 1.0)
                        nc.sync.dma_start(Vall3[:, :, 0:HD], v_src[ti])

                # --- qT/kT [64, j*128 + t*16 + h] via PE transposes ---
                # (group-interleaved columns so the scores matmul operands
                # are plain contiguous slices: walrus allows only one free
                # dim on the matmul RHS AP)
                qkT = {}
                for n, drain in (("Wq", nc.scalar.copy),
                                 ("Wk", nc.vector.tensor_copy)):
                    dstT = tp_sb.tile([64, NH * 128], BF, tag=f"{n}T",
                                      name=f"{n}T")
                    dview = dstT.rearrange("p (j t h) -> p h j t", t=8, h=NH)
                    for hq in range(2):
                        ps = tps.tile([64, 1024], BF, tag="tp", name="tp")
                        for hl in range(8):
                            h = hq * 8 + hl
                            nc.tensor.transpose(
                                ps[:, hl * 128:(hl + 1) * 128],
                                sb[n][:, h * HD:(h + 1) * HD], ident[:, :])
                        drain(dview[:, hq * 8:(hq + 1) * 8, :, :],
                              ps.rearrange("p (h j t) -> p h j t", j=NG, t=8))
                    qkT[n] = dstT
                qTg = qkT["Wq"]
                kTg = qkT["Wk"]

                # --- attention: scores phase, then attn phase ---
                u_all = up.tile([128, NG * HD], BF, tag="uall", name="uall")
                u3 = u_all.rearrange("p (j d) -> p j d", d=HD)
                em4s = []
                for jq in range(4):
                    ps1 = sps.tile([128, 512], F32, tag="ps", name="ps1")
                    for jl in range(4):
                        j = jq * 4 + jl
                        nc.tensor.matmul(ps1[:, jl * 128:(jl + 1) * 128],
                                         kTg[:, j * 128:(j + 1) * 128],
                                         qTg[:, j * 128:(j + 1) * 128],
                                         start=True, stop=True)
                    e4 = ap_.tile([128, 512], BF, tag="e", name="e4")
                    nc.scalar.activation(e4[:, :], ps1[:, :], AF.Exp,
                                         scale=0.125)
                    em4 = ep_.tile([128, 512], BF, tag="em", name="em4")
                    nc.gpsimd.tensor_tensor(em4[:, :], e4[:, :], mask4[:, :],
                                            OP.mult)
                    em4s.append(em4)
                for jq in range(4):
                    em4 = em4s[jq]
                    ps2 = sps.tile([128, 512], F32, tag="ps", name="ps2")
                    ps2v = ps2[:, 0:4 * (HD + 1)].rearrange(
                        "p (j c) -> p j c", c=HD + 1)
                    for jl in range(4):
                        j = jq * 4 + jl
                        nc.tensor.matmul(ps2v[:, jl, :],
                                         em4[:, jl * 128:(jl + 1) * 128],
                                         Vall3[:, j, :],
                                         start=True, stop=True)
                    rec4 = ap_.tile([128, 4], F32, tag="rec", name="rec4")
                    rec4v = rec4.rearrange("p (j c) -> p j c", c=1)
                    nc.vector.reciprocal(rec4v[:, :, :], ps2v[:, :, HD:HD + 1])
                    nc.vector.tensor_tensor(
                        u3[:, jq * 4:(jq + 1) * 4, :], ps2v[:, :, 0:HD],
                        rec4v.broadcast_to([128, 4, HD]), OP.mult)

                # keep the u write on the SP ring: the uT transpose reads
                # below rely on same-ring ordering with it (cross-ring
                # DMA-DMA RAW through DRAM raced on HW)
                nc.sync.dma_start(u_dst[ti],
                                  u_all.rearrange("p (j d) -> p j d", d=HD))

            def issue_uT(gr):
                uT = op_.tile([128, IC * 512], BF, tag="uT", name="uT")
                for ic in range(IC):
                    nc.sync.dma_start_transpose(
                        uT[:, ic * 512:(ic + 1) * 512],
                        ubf[gr * 512:(gr + 1) * 512,
                            ic * 128:(ic + 1) * 128])
                uT_tiles[gr] = uT

            def issue_uT_tile(tj):
                uTt = op_.tile([128, IC * 128], BF, tag="uTt", name="uTt")
                for ic in range(IC):
                    nc.sync.dma_start_transpose(
                        uTt[:, ic * 128:(ic + 1) * 128],
                        ubf[tj * 128:(tj + 1) * 128, ic * 128:(ic + 1) * 128])
                uTt_tiles[tj] = uTt

            def yproj_tile(tj):
                uTt = uTt_tiles.pop(tj)
                y_sb = op_.tile([128, HID], F32, tag="ysb", name="y_sb")
                for hf in range(2):
                    yp = yps.tile([128, 512], F32, tag="yproj", name="yproj")
                    for ic in range(IC):
                        nc.tensor.matmul(
                            yp[:, :], uTt[:, ic * 128:(ic + 1) * 128],
                            wt["Wo"][:, ic * HID + hf * 512:
                                     ic * HID + (hf + 1) * 512],
                            start=(ic == 0), stop=(ic == IC - 1))
                    nc.vector.tensor_tensor(
                        y_sb[:, hf * 512:(hf + 1) * 512], yp[:, :],
                        btile["bo"][:, hf * 512:(hf + 1) * 512], OP.add)
                nc.sync.dma_start(y[tj * 128:(tj + 1) * 128, :], y_sb[:, :])

            main_loop()

            # tail: pad the SP queue between the final u write and the uT
            # transposes so the write's transfer completes first
            for pad in range(2):
                dummy = op_.tile([128, 256], BF, tag="dummy", name="dummy")
                nc.sync.dma_start(dummy[:, :],
                                  vdram[pad * 128:(pad + 1) * 128, 0:256])
            issue_uT_tile(NT - 1)
            yproj_tile(NT - 2)
            yproj_tile(NT - 1)

    nc.finalize()
    return nc


def make_in_maps(x, Wq, bq, Wk, bk, Wv, bv, Wo, bo):
    """Host-side marshalling: shard x over cores, pre-transpose + bf16-cast
    x and the projection weights (device reads them directly into SBUF)."""
    x2 = np.asarray(x, dtype=np.float32).reshape(T_FULL, HID)
    ident = np.eye(128, dtype=np.float32).astype(NP_BF16)
    blk = (np.arange(128)[:, None] // 16 == np.arange(128)[None, :] // 16)
    mask4 = np.tile(blk.astype(np.float32), (1, 4)).astype(NP_BF16)
    full = {
        "ident": ident,
        "mask4": np.ascontiguousarray(mask4),
        "WqT": np.ascontiguousarray(np.asarray(Wq, np.float32).T).astype(NP_BF16),
        "WkT": np.ascontiguousarray(np.asarray(Wk, np.float32).T).astype(NP_BF16),
        "WvT": np.ascontiguousarray(np.asarray(Wv, np.float32).T).astype(NP_BF16),
        "WoT": np.ascontiguousarray(np.asarray(Wo, np.float32).T).astype(NP_BF16),
        "bq": np.ascontiguousarray(bq, dtype=np.float32),
        "bk": np.ascontiguousarray(bk, dtype=np.float32),
        "bv": np.ascontiguousarray(bv, dtype=np.float32),
        "bo": np.ascontiguousarray(bo, dtype=np.float32),
    }
    in_maps = []
    for c in range(NCORES):
        m = dict(full)
        m["xT"] = np.ascontiguousarray(x2[c * T:(c + 1) * T].T).astype(NP_BF16)
        in_maps.append(m)
    return in_maps


def kernel(x, Wq, bq, Wk, bk, Wv, bv, Wo, bo, **_unused):
    global _cached
    if _cached is None:
        _cached = build_program()
    nc = _cached

    in_maps = make_in_maps(x, Wq, bq, Wk, bk, Wv, bv, Wo, bo)
    res = run_bass_kernel_spmd(nc, in_maps, core_ids=list(range(NCORES)))
    out = np.concatenate([r["y"] for r in res.results], axis=0)
    return out.reshape(B, S, HID).astype(np.float32)


if __name__ == "__main__":
    rng = np.random.default_rng(0)
    ins = {k: rng.standard_normal(v, dtype=np.float32) * (0.02 if k[0] in "Wb" else 1.0)
           for k, v in [("x", (B, S, HID)), ("Wq", (HID, HID)), ("bq", (HID,)),
                        ("Wk", (HID, HID)), ("bk", (HID,)), ("Wv", (HID, HID)),
                        ("bv", (HID,)), ("Wo", (HID, HID)), ("bo", (HID,))]}
    out = kernel(**ins)
    print(out.shape, out.dtype)
